# revision 9
# baseline (speedup 1.0000x reference)
"""Trainium2 Bass kernel for nn_LinearEmbedded (moe_routing).

Reference computation:
    w = weight1[region_ix]             # (B, C, D) gather per-region weights
    out = einsum('abc,bcd->abd', x, w) + bias1[region_ix][None]

Shapes: x (A=128, B=128, C=512), weight1 (1000, 512, 512), bias1 (1000, 512).

Sharding: the B axis (regions/minibatch) is split across 8 NeuronCores,
16 regions per core.  The per-region weight/bias gather happens host-side
(each core only receives the 16 gathered weight slices it needs), so the
device kernel is a dense batched matmul:

    per b in 0..15:  out[b] = x_b @ w_b + bias_b     # [128,512]@[512,512]

Matmul operands are cast to fp16 host-side: halves the weight-DMA (the
binding roofline) and the PE streams fp16 at 1 cycle/row vs fp32's 2x
half-rate passes.  PSUM accumulation stays fp32; measured end-to-end
l2 relative error vs the fp32 reference is ~3e-4 (resid_var ~9e-8).

Device layout per core (host pre-transposed so every DMA is contiguous):
    xt   (16, 128, 4, 128) f16  xt[b,p,k,a] = x[a, B0+b, 128k+p]  (lhsT)
    w    (16, 128, 4, 512) f16  w[b,p,k,n]  = weight1[r_b][128k+p, n] (rhs)
    bias (1, 16*512+128)   f16  biases + trailing 128 ones (bias-matmul lhsT)
    out  (16, 128, 512)    f32  out[b,a,n]

Per b: 4 accumulating K=128 matmuls + a K=1 ones-x-bias matmul into one
PSUM bank, DVE copy to SBUF, contiguous store.  Weight loads are split
(4 chunks for b=0, halves after) so the first matmul starts as early as
possible and PE stalls stay short; xt/out ride the ACT HWDGE ring while
w/bias ride the SP ring.
"""

import numpy as np

A, B, C, D = 128, 128, 512, 512
NCORES = 8
BL = B // NCORES  # 16 regions per core
KC = C // 128     # 4 contraction chunks

_prog = None


def _build_program():
    global _prog
    if _prog is not None:
        return _prog

    import concourse.bacc as bacc
    import concourse.mybir as mybir
    import concourse.tile as tile

    F32 = mybir.dt.float32
    F16 = mybir.dt.float16
    nc = bacc.Bacc("TRN2", target_bir_lowering=False, debug=False)
    xt = nc.dram_tensor("xt", [BL, 128, KC, A], F16, kind="ExternalInput")
    w = nc.dram_tensor("w", [BL, 128, KC, D], F16, kind="ExternalInput")
    bias = nc.dram_tensor("bias", [1, BL * D + A], F16, kind="ExternalInput")
    out = nc.dram_tensor("out", [BL, A, D], F32, kind="ExternalOutput")

    with tile.TileContext(nc) as tc:
        with (
            tc.tile_pool(name="cst", bufs=1) as cst,
            tc.tile_pool(name="xp", bufs=6) as xp,
            tc.tile_pool(name="wp0", bufs=4) as wp0,
            tc.tile_pool(name="wph", bufs=6) as wph,
            tc.tile_pool(name="op", bufs=3) as op,
            tc.tile_pool(name="ps", bufs=4, space="PSUM") as ps,
        ):
            bias_t = cst.tile([1, BL * D + A], F16)
            ones = bias_t[:, BL * D : BL * D + A]

            # HAM warmup: ~3.5us of dummy matmuls on a zeroed tile during
            # the head (while the first weight DMAs stream in) so the PE
            # clock-gate releases (1.2 -> 2.4 GHz) before real work arrives.
            warm = cst.tile([128, D], F16)
            nc.gpsimd.memset(warm[:], 0.0)
            psw = ps.tile([A, D], F32, tag="warm")
            for _ in range(9):
                nc.tensor.matmul(psw[:], warm[:, 0:128], warm[:], start=True, stop=True)

            for b in range(BL):
                xtile = xp.tile([128, KC, A], F16)
                nc.scalar.dma_start(xtile[:], xt[b, :, :, :])

                if b == 0:
                    wk = []
                    for k in range(KC):
                        t = wp0.tile([128, D], F16, tag=f"w0_{k}")
                        nc.sync.dma_start(t[:], w[0, :, k, :])
                        wk.append(t)
                    # bias is only needed by b=0's 5th matmul; emit after
                    # the critical first weight chunks.
                    nc.sync.dma_start(bias_t[:], bias[:])
                    rhs = [wk[k][:, :] for k in range(KC)]
                else:
                    wha = wph.tile([128, 2, D], F16, tag="wha")
                    whb = wph.tile([128, 2, D], F16, tag="whb")
                    nc.sync.dma_start(wha[:], w[b, :, 0:2, :])
                    nc.sync.dma_start(whb[:], w[b, :, 2:4, :])
                    rhs = [wha[:, 0, :], wha[:, 1, :], whb[:, 0, :], whb[:, 1, :]]

                psum = ps.tile([A, D], F32)
                for k in range(KC):
                    nc.tensor.matmul(
                        psum[:],
                        xtile[:, k, :],
                        rhs[k],
                        start=(k == 0),
                        stop=False,
                    )
                nc.tensor.matmul(
                    psum[:],
                    ones,
                    bias_t[:, b * D : (b + 1) * D],
                    start=False,
                    stop=True,
                )

                otile = op.tile([A, D], F32)
                nc.vector.tensor_copy(otile[:], psum[:])
                nc.scalar.dma_start(out[b, :, :], otile[:])

    nc.compile()
    _prog = nc
    return nc


def _shard_inputs(x, region_ix, weight1, bias1):
    x16 = x.astype(np.float16)
    in_maps = []
    for c in range(NCORES):
        bs = slice(c * BL, (c + 1) * BL)
        rloc = region_ix[bs]
        # xt[b,p,k,a] = x[a, B0+b, 128k+p]
        xs = x16[:, bs, :].transpose(1, 2, 0)  # (BL, C, A)
        xt = np.ascontiguousarray(
            xs.reshape(BL, KC, 128, A).transpose(0, 2, 1, 3)
        )
        wg = weight1[rloc].astype(np.float16)  # (BL, C, D)
        wdev = np.ascontiguousarray(
            wg.reshape(BL, KC, 128, D).transpose(0, 2, 1, 3)
        )
        bg = np.concatenate(
            [bias1[rloc].astype(np.float16).reshape(BL * D), np.ones(A, np.float16)]
        ).reshape(1, BL * D + A)
        in_maps.append({"xt": xt, "w": wdev, "bias": bg})
    return in_maps


def kernel(x, region_ix, weight1, bias1):
    from concourse.bass_utils import run_bass_kernel_spmd

    x = np.asarray(x, dtype=np.float32)
    region_ix = np.asarray(region_ix).astype(np.int64)
    weight1 = np.asarray(weight1, dtype=np.float32)
    bias1 = np.asarray(bias1, dtype=np.float32)

    nc = _build_program()
    in_maps = _shard_inputs(x, region_ix, weight1, bias1)
    res = run_bass_kernel_spmd(nc, in_maps, core_ids=list(range(NCORES)))

    out = np.empty((A, B, D), dtype=np.float32)
    for c in range(NCORES):
        out[:, c * BL : (c + 1) * BL, :] = res.results[c]["out"].transpose(1, 0, 2)
    return out


# revision 11
# speedup vs baseline: 1.1941x; 1.1941x over previous
"""Trainium2 Bass kernel for nn_LinearEmbedded (moe_routing).

Reference computation:
    w = weight1[region_ix]             # (B, C, D) gather per-region weights
    out = einsum('abc,bcd->abd', x, w) + bias1[region_ix][None]

Shapes: x (A=128, B=128, C=512), weight1 (1000, 512, 512), bias1 (1000, 512).

Sharding: the B axis (regions/minibatch) is split across 8 NeuronCores,
16 regions per core.  The per-region weight/bias gather happens host-side
(each core only receives the 16 gathered weight slices it needs), so the
device kernel is a dense batched matmul:

    per b in 0..15:  out[b] = x_b @ w_b + bias_b     # [128,512]@[512,512]

Matmul operands are cast to fp16 host-side: halves the weight-DMA (the
binding roofline) and the PE streams fp16 at 1 cycle/row vs fp32's 2x
half-rate passes.  PSUM accumulation stays fp32; measured end-to-end
l2 relative error vs the fp32 reference is ~3e-4 (resid_var ~9e-8).

Device layout per core (host pre-transposed so every DMA is contiguous):
    xt   (16, 128, 4, 128) f16  xt[b,p,k,a] = x[a, B0+b, 128k+p]  (lhsT)
    w    (16, 128, 4, 512) f16  w[b,p,k,n]  = weight1[r_b][128k+p, n] (rhs)
    bias (1, 16*512+128)   f16  biases + trailing 128 ones (bias-matmul lhsT)
    out  (16, 128, 512)    f32  out[b,a,n]

Per b: 4 accumulating K=128 matmuls + a K=1 ones-x-bias matmul into one
PSUM bank, DVE copy to SBUF, contiguous store.  Weight loads are split
(4 chunks for b=0, halves after) so the first matmul starts as early as
possible and PE stalls stay short; xt/out ride the ACT HWDGE ring while
w/bias ride the SP ring.
"""

import numpy as np

A, B, C, D = 128, 128, 512, 512
NCORES = 8
BL = B // NCORES  # 16 regions per core
KC = C // 128     # 4 contraction chunks

_prog = None


def _build_program():
    global _prog
    if _prog is not None:
        return _prog

    import concourse.bacc as bacc
    import concourse.mybir as mybir
    import concourse.tile as tile

    F32 = mybir.dt.float32
    F16 = mybir.dt.float16
    nc = bacc.Bacc("TRN2", target_bir_lowering=False, debug=False)
    xt = nc.dram_tensor("xt", [BL, 128, KC, A], F16, kind="ExternalInput")
    w = nc.dram_tensor("w", [BL, 128, KC, D], F16, kind="ExternalInput")
    bias = nc.dram_tensor("bias", [1, BL * D + A], F16, kind="ExternalInput")
    out = nc.dram_tensor("out", [BL, A, D], F32, kind="ExternalOutput")

    with tile.TileContext(nc) as tc:
        with (
            tc.tile_pool(name="cst", bufs=1) as cst,
            tc.tile_pool(name="xp", bufs=6) as xp,
            tc.tile_pool(name="wp0", bufs=4) as wp0,
            tc.tile_pool(name="wph", bufs=6) as wph,
            tc.tile_pool(name="op", bufs=3) as op,
            tc.tile_pool(name="ps", bufs=4, space="PSUM") as ps,
        ):
            bias_t = cst.tile([1, BL * D + A], F16)
            ones = bias_t[:, BL * D : BL * D + A]

            for b in range(BL):
                # SWDGE ring: keeps xt prefetch off the ACT ring, where the
                # copy-gated out[b] dispatches would head-of-line block it.
                xtile = xp.tile([128, KC, A], F16)
                nc.gpsimd.dma_start(xtile[:], xt[b, :, :, :])

                if b == 0:
                    wk = []
                    for k in range(KC):
                        t = wp0.tile([128, D], F16, tag=f"w0_{k}")
                        nc.sync.dma_start(t[:], w[0, :, k, :])
                        wk.append(t)
                    # bias is only needed by b=0's 5th matmul; emit after
                    # the critical first weight chunks.
                    nc.sync.dma_start(bias_t[:], bias[:])
                    rhs = [wk[k][:, :] for k in range(KC)]
                else:
                    wha = wph.tile([128, 2, D], F16, tag="wha")
                    whb = wph.tile([128, 2, D], F16, tag="whb")
                    nc.sync.dma_start(wha[:], w[b, :, 0:2, :])
                    nc.sync.dma_start(whb[:], w[b, :, 2:4, :])
                    rhs = [wha[:, 0, :], wha[:, 1, :], whb[:, 0, :], whb[:, 1, :]]

                psum = ps.tile([A, D], F32)
                for k in range(KC):
                    nc.tensor.matmul(
                        psum[:],
                        xtile[:, k, :],
                        rhs[k],
                        start=(k == 0),
                        stop=False,
                    )
                nc.tensor.matmul(
                    psum[:],
                    ones,
                    bias_t[:, b * D : (b + 1) * D],
                    start=False,
                    stop=True,
                )

                otile = op.tile([A, D], F32)
                nc.vector.tensor_copy(otile[:], psum[:])
                nc.scalar.dma_start(out[b, :, :], otile[:])

    nc.compile()
    _prog = nc
    return nc


def _shard_inputs(x, region_ix, weight1, bias1):
    x16 = x.astype(np.float16)
    in_maps = []
    for c in range(NCORES):
        bs = slice(c * BL, (c + 1) * BL)
        rloc = region_ix[bs]
        # xt[b,p,k,a] = x[a, B0+b, 128k+p]
        xs = x16[:, bs, :].transpose(1, 2, 0)  # (BL, C, A)
        xt = np.ascontiguousarray(
            xs.reshape(BL, KC, 128, A).transpose(0, 2, 1, 3)
        )
        wg = weight1[rloc].astype(np.float16)  # (BL, C, D)
        wdev = np.ascontiguousarray(
            wg.reshape(BL, KC, 128, D).transpose(0, 2, 1, 3)
        )
        bg = np.concatenate(
            [bias1[rloc].astype(np.float16).reshape(BL * D), np.ones(A, np.float16)]
        ).reshape(1, BL * D + A)
        in_maps.append({"xt": xt, "w": wdev, "bias": bg})
    return in_maps


def kernel(x, region_ix, weight1, bias1):
    from concourse.bass_utils import run_bass_kernel_spmd

    x = np.asarray(x, dtype=np.float32)
    region_ix = np.asarray(region_ix).astype(np.int64)
    weight1 = np.asarray(weight1, dtype=np.float32)
    bias1 = np.asarray(bias1, dtype=np.float32)

    nc = _build_program()
    in_maps = _shard_inputs(x, region_ix, weight1, bias1)
    res = run_bass_kernel_spmd(nc, in_maps, core_ids=list(range(NCORES)))

    out = np.empty((A, B, D), dtype=np.float32)
    for c in range(NCORES):
        out[:, c * BL : (c + 1) * BL, :] = res.results[c]["out"].transpose(1, 0, 2)
    return out
